# revision 12
# baseline (speedup 1.0000x reference)
"""Linformer attention TRN2 kernel (8 NeuronCores).

Sharding: core c handles batch b = c//2 and head-half hh = c%2
(8 of 16 heads = 512 of 1024 feature columns of Wq/Wk/Wv, and the
matching 512 rows of Wo). Each core computes a partial output
y_part = attn_out_half @ Wo[hh*512:(hh+1)*512, :]; the host sums the
two partials per batch and adds bo.

Per-core dataflow (L=4096, D=1024, 8 heads x hd=64, K=256):
  phase 1 (stream L in slices of 256):
    xT   = transpose(x_slice)                       (PE transpose)
    qT  += Wq_s^T @ xT       -> [512, L]  (SBUF-resident, bias folded)
    k,v  = xT^T @ Wk_s/Wv_s  -> [Lslice, 512] (transient, bias folded)
    kET += k_headpair^T @ E  -> [128(2 heads), 256] x4   (PSUM-resident)
    vF  += F_chunk^T @ v     -> [128(Kchunk), 512(8 heads)] x2 (PSUM)
  phase 2 (per L-tile of 512, per head):
    scores = qT_h^T @ kET_h            [128L, 256]   (PSUM)
    m = rowmax; s = exp(scale*scores - scale*m), Z = rowsum (ACT accum)
    s = exp(scale*scores - scale*m - ln Z)           (normalized)
    sT = PE-transpose(s)               [256, 512L]
    outT_h = vF_h^T @ sT               [64, 512L]
    y_tile = outT^T @ Wo_s             [128L, 1024] -> DRAM
"""

import sys

sys.path.insert(0, "/opt/trn_rl_repo")

import numpy as np

import concourse.bass as bass
import concourse.mybir as mybir
import concourse.tile as tile
from concourse import bacc
from concourse import bass_utils

B, L, D, H, HD, K = 4, 4096, 1024, 16, 64, 256
DH = 512                      # per-core feature slice (8 heads x 64)
NHL = 8                       # heads per core
SCALE = HD ** -0.5
P = 128
LS1 = 256                     # phase-1 L slice
NS1 = L // LS1                # 16 phase-1 iterations
LT2 = 512                     # phase-2 L tile
NT2 = L // LT2                # 8 phase-2 iterations
F32 = mybir.dt.float32
F32R = mybir.dt.float32r

USE_F32R = True               # fast fp32 matmul mode for GEMM operands
MMDT = F32R if USE_F32R else F32   # dtype of every matmul-feeding tensor

_CACHE = {}


def _r(ap):
    return ap


def _rt(ap):
    return ap


def build_program():
    nc = bacc.Bacc("TRN2", target_bir_lowering=False, debug=False)

    xb = nc.dram_tensor("xb", [L, D], MMDT, kind="ExternalInput").ap()
    wq = nc.dram_tensor("wq", [D, DH], MMDT, kind="ExternalInput").ap()
    wk = nc.dram_tensor("wk", [D, DH], MMDT, kind="ExternalInput").ap()
    wv = nc.dram_tensor("wv", [D, DH], MMDT, kind="ExternalInput").ap()
    wo = nc.dram_tensor("wo", [DH, D], MMDT, kind="ExternalInput").ap()
    bqc = nc.dram_tensor("bqc", [P, DH // P], F32, kind="ExternalInput").ap()
    bkr = nc.dram_tensor("bkr", [P, DH], F32, kind="ExternalInput").ap()
    bvr = nc.dram_tensor("bvr", [P, DH], F32, kind="ExternalInput").ap()
    Edr = nc.dram_tensor("E", [L, K], MMDT, kind="ExternalInput").ap()
    Fdr = nc.dram_tensor("F", [L, K], MMDT, kind="ExternalInput").ap()
    idr = nc.dram_tensor("ident", [P, P], MMDT, kind="ExternalInput").ap()
    ydr = nc.dram_tensor("y", [L, D], F32, kind="ExternalOutput").ap()

    with tile.TileContext(nc) as tc:
        with (
            tc.tile_pool(name="const", bufs=1) as constp,
            tc.tile_pool(name="persist", bufs=1) as persist,
        ):
            ident_t = constp.tile([P, P], MMDT, name="ident_t")
            nc.sync.dma_start(ident_t[:], idr)
            ident = ident_t[:]
            bqc_sb = constp.tile([P, DH // P], F32)
            nc.sync.dma_start(bqc_sb[:], bqc)
            bkr_sb = constp.tile([P, DH], F32)
            nc.sync.dma_start(bkr_sb[:], bkr)
            bvr_sb = constp.tile([P, DH], F32)
            nc.sync.dma_start(bvr_sb[:], bvr)

            # persistent SBUF tensors
            qT = persist.tile([P, 4, L], MMDT)          # qT[c*128+p, l]
            kET_sb = persist.tile([P, 4, K], MMDT)      # [2-head pair rows, pair, k]
            vF_sb = persist.tile([P, 2, DH], MMDT)      # [k within chunk, kchunk, h*64+d]

            # ---------------- phase 1 ----------------
            with (
                tc.tile_pool(name="w1", bufs=1) as w1,
                tc.tile_pool(name="xin", bufs=2) as xin,
                tc.tile_pool(name="xtp", bufs=2) as xtp,
                tc.tile_pool(name="kvp", bufs=2) as kvp,
                tc.tile_pool(name="efp", bufs=2) as efp,
                tc.tile_pool(name="ps_acc", bufs=1, space="PSUM") as ps_acc,
                tc.tile_pool(name="ps_tp1", bufs=2, space="PSUM") as ps_tp1,
                tc.tile_pool(name="ps_mm1", bufs=2, space="PSUM") as ps_mm1,
            ):
                wq_sb = w1.tile([P, D // P, DH], MMDT, tag="wq")
                nc.sync.dma_start(wq_sb[:], wq.rearrange("(c p) n -> p c n", p=P))
                wk_sb = w1.tile([P, D // P, DH], MMDT, tag="wk")
                nc.sync.dma_start(wk_sb[:], wk.rearrange("(c p) n -> p c n", p=P))
                wv_sb = w1.tile([P, D // P, DH], MMDT, tag="wv")
                nc.sync.dma_start(wv_sb[:], wv.rearrange("(c p) n -> p c n", p=P))

                # persistent PSUM accumulators
                kET_ps = [
                    ps_acc.tile([P, 2 * K], F32, tag=f"kET{i}", name=f"kET_ps{i}")
                    for i in range(2)
                ]
                vF_ps = [
                    ps_acc.tile([P, DH], F32, tag=f"vF{i}", name=f"vF_ps{i}")
                    for i in range(2)
                ]

                for ls in range(NS1):
                    l0 = ls * LS1
                    x_sl = xin.tile([P, LS1 // P, D], MMDT)
                    nc.sync.dma_start(
                        x_sl[:], xb[l0 : l0 + LS1, :].rearrange("(a p) d -> p a d", p=P)
                    )
                    # transpose x slice -> xT [d, l]  (tile [128, 8 dchunks, 256])
                    xT = xtp.tile([P, D // P, LS1], MMDT)
                    for a in range(LS1 // P):
                        for g in range(2):  # 2 groups of 4 dchunks
                            tp = ps_tp1.tile([P, 4 * P], MMDT, tag="tp")
                            for dd in range(4):
                                dc = g * 4 + dd
                                nc.tensor.transpose(
                                    tp[:, dd * P : (dd + 1) * P],
                                    _rt(x_sl[:, a, dc * P : (dc + 1) * P]),
                                    _rt(ident),
                                )
                            # copy the 4 transposed [128,128] blocks into xT
                            nc.scalar.copy(
                                xT[:, g * 4 : (g + 1) * 4, a * P : (a + 1) * P],
                                tp[:].rearrange("p (c n) -> p c n", n=P),
                            )
                    # qT chunks: lhsT = wq chunk, rhs = xT
                    for rc in range(4):
                        pq = ps_mm1.tile([P, LS1], F32, tag="mm1")
                        for dc in range(D // P):
                            nc.tensor.matmul(
                                pq[:],
                                _r(wq_sb[:, dc, rc * P : (rc + 1) * P]),
                                _r(xT[:, dc, :]),
                                start=(dc == 0),
                                stop=(dc == D // P - 1),
                            )
                        nc.vector.tensor_scalar_add(
                            qT[:, rc, l0 : l0 + LS1], pq[:], bqc_sb[:, rc : rc + 1]
                        )
                    # k, v natural layout slices
                    k_sl = kvp.tile([P, LS1 // P, DH], MMDT, tag="k")
                    v_sl = kvp.tile([P, LS1 // P, DH], MMDT, tag="v")
                    for a in range(LS1 // P):
                        pk = ps_mm1.tile([P, DH], F32, tag="mm1")
                        for dc in range(D // P):
                            nc.tensor.matmul(
                                pk[:],
                                _r(xT[:, dc, a * P : (a + 1) * P]),
                                _r(wk_sb[:, dc, :]),
                                start=(dc == 0),
                                stop=(dc == D // P - 1),
                            )
                        nc.vector.tensor_add(k_sl[:, a, :], pk[:], bkr_sb[:])
                        pv = ps_mm1.tile([P, DH], F32, tag="mm1")
                        for dc in range(D // P):
                            nc.tensor.matmul(
                                pv[:],
                                _r(xT[:, dc, a * P : (a + 1) * P]),
                                _r(wv_sb[:, dc, :]),
                                start=(dc == 0),
                                stop=(dc == D // P - 1),
                            )
                        nc.vector.tensor_add(v_sl[:, a, :], pv[:], bvr_sb[:])
                    # E / F slices
                    e_sl = efp.tile([P, LS1 // P, K], MMDT, tag="E")
                    nc.sync.dma_start(
                        e_sl[:], Edr[l0 : l0 + LS1, :].rearrange("(a p) k -> p a k", p=P)
                    )
                    f_sl = efp.tile([P, LS1 // P, K], MMDT, tag="F")
                    nc.sync.dma_start(
                        f_sl[:], Fdr[l0 : l0 + LS1, :].rearrange("(a p) k -> p a k", p=P)
                    )
                    first = ls == 0
                    last = ls == NS1 - 1
                    for a in range(LS1 // P):
                        # kET accumulation: 4 head-pairs, 2 pairs share a
                        # PSUM bank -> only one start/stop per bank (zero
                        # region); first write per element overwrites.
                        for pr in range(4):
                            nc.tensor.matmul(
                                kET_ps[pr // 2][:, (pr % 2) * K : (pr % 2 + 1) * K],
                                _r(k_sl[:, a, pr * P : (pr + 1) * P]),
                                _r(e_sl[:, a, :]),
                                start=(first and a == 0 and pr % 2 == 0),
                                stop=(last and a == LS1 // P - 1 and pr % 2 == 1),
                            )
                        # vF accumulation: 2 K-chunks
                        for kc in range(2):
                            nc.tensor.matmul(
                                vF_ps[kc][:],
                                _r(f_sl[:, a, kc * P : (kc + 1) * P]),
                                _r(v_sl[:, a, :]),
                                start=(first and a == 0),
                                stop=(last and a == LS1 // P - 1),
                            )
                # copy accumulators to SBUF
                for i in range(2):
                    nc.scalar.copy(
                        kET_sb[:, 2 * i : 2 * i + 2, :],
                        kET_ps[i][:].rearrange("p (c n) -> p c n", n=K),
                    )
                    nc.scalar.copy(vF_sb[:, i, :], vF_ps[i][:])

            # ---------------- phase 2 ----------------
            with (
                tc.tile_pool(name="w2", bufs=1) as w2,
                tc.tile_pool(name="sp", bufs=3) as sp,
                tc.tile_pool(name="stp", bufs=2) as stp,
                tc.tile_pool(name="otp", bufs=2) as otp,
                tc.tile_pool(name="obp", bufs=2) as obp,
                tc.tile_pool(name="yp", bufs=2) as yp,
                tc.tile_pool(name="stat", bufs=8) as stat,
                tc.tile_pool(name="ps_s", bufs=2, space="PSUM") as ps_s,
                tc.tile_pool(name="ps_st", bufs=2, space="PSUM") as ps_st,
                tc.tile_pool(name="ps_pv", bufs=2, space="PSUM") as ps_pv,
                tc.tile_pool(name="ps_y", bufs=2, space="PSUM") as ps_y,
            ):
                wo_sb = w2.tile([P, DH // P, D], MMDT)
                nc.sync.dma_start(wo_sb[:], wo.rearrange("(c p) n -> p c n", p=P))

                for lt in range(NT2):
                    l0 = lt * LT2
                    outT = otp.tile([P, 4, LT2], MMDT)
                    for h in range(NHL):
                        po = (h % 2) * HD          # partition offset within pair
                        pair = h // 2
                        sT = stp.tile([P, 2, LT2], MMDT)
                        for j in range(LT2 // P):
                            ps = ps_s.tile([P, K], F32)
                            nc.tensor.matmul(
                                ps[:],
                                _r(qT[po : po + HD, pair, l0 + j * P : l0 + (j + 1) * P]),
                                _r(kET_sb[po : po + HD, pair, :]),
                                start=True,
                                stop=True,
                            )
                            m = stat.tile([P, 1], F32, tag="m")
                            nc.vector.reduce_max(m[:], ps[:], axis=mybir.AxisListType.X)
                            negm = stat.tile([P, 1], F32, tag="negm")
                            nc.vector.tensor_scalar_mul(negm[:], m[:], -SCALE)
                            z = stat.tile([P, 1], F32, tag="z")
                            s = sp.tile([P, K], MMDT)
                            nc.scalar.activation(
                                s[:],
                                ps[:],
                                mybir.ActivationFunctionType.Exp,
                                bias=negm[:],
                                scale=SCALE,
                                accum_out=z[:],
                            )
                            rz = stat.tile([P, 1], F32, tag="rz")
                            nc.vector.reciprocal(rz[:], z[:])
                            nc.gpsimd.tensor_scalar_mul(s[:], s[:], rz[:])
                            # transpose s [128, 256] -> two [128, 128] blocks
                            pt = ps_st.tile([P, 2 * P], MMDT, tag="st")
                            for kc in range(2):
                                nc.tensor.transpose(
                                    pt[:, kc * P : (kc + 1) * P],
                                    _rt(s[:, kc * P : (kc + 1) * P]),
                                    _rt(ident),
                                )
                            eng = nc.vector if j % 2 == 0 else nc.scalar
                            if j % 2 == 0:
                                nc.vector.tensor_copy(
                                    sT[:, :, j * P : (j + 1) * P],
                                    pt[:].rearrange("p (c n) -> p c n", n=P),
                                )
                            else:
                                nc.scalar.copy(
                                    sT[:, :, j * P : (j + 1) * P],
                                    pt[:].rearrange("p (c n) -> p c n", n=P),
                                )
                        # PV: outT_h [64, LT2] (PSUM dst must start at partition 0)
                        pp = ps_pv.tile([HD, LT2], F32)
                        for kc in range(2):
                            nc.tensor.matmul(
                                pp[:],
                                _r(vF_sb[:, kc, h * HD : (h + 1) * HD]),
                                _r(sT[:, kc, :]),
                                start=(kc == 0),
                                stop=(kc == 1),
                            )
                        if po == 0:
                            nc.scalar.copy(outT[0:HD, pair, :], pp[:])
                        else:
                            ob = obp.tile([HD, LT2], MMDT, name="obounce")
                            nc.scalar.copy(ob[:], pp[:])
                            nc.sync.dma_start(outT[po : po + HD, pair, :], ob[:])
                    # y = outT^T @ Wo_s
                    for j in range(LT2 // P):
                        y_sb = yp.tile([P, D], F32)
                        for nh in range(2):
                            py = ps_y.tile([P, DH], F32)
                            for c in range(4):
                                nc.tensor.matmul(
                                    py[:],
                                    _r(outT[:, c, j * P : (j + 1) * P]),
                                    _r(wo_sb[:, c, nh * DH : (nh + 1) * DH]),
                                    start=(c == 0),
                                    stop=(c == 3),
                                )
                            nc.scalar.copy(y_sb[:, nh * DH : (nh + 1) * DH], py[:])
                        nc.sync.dma_start(ydr[l0 + j * P : l0 + (j + 1) * P, :], y_sb[:])
    nc.compile()
    return nc


def _get_program():
    if "nc" not in _CACHE:
        _CACHE["nc"] = build_program()
    return _CACHE["nc"]


def _shard_inputs(inputs):
    x = np.asarray(inputs["x"], np.float32)
    Wq = np.asarray(inputs["Wq"], np.float32)
    bq = np.asarray(inputs["bq"], np.float32)
    Wk = np.asarray(inputs["Wk"], np.float32)
    bk = np.asarray(inputs["bk"], np.float32)
    Wv = np.asarray(inputs["Wv"], np.float32)
    bv = np.asarray(inputs["bv"], np.float32)
    E = np.ascontiguousarray(np.asarray(inputs["E"], np.float32))
    F = np.ascontiguousarray(np.asarray(inputs["F"], np.float32))
    Wo = np.asarray(inputs["Wo"], np.float32)
    in_maps = []
    for c in range(8):
        b, hh = c // 2, c % 2
        sl = slice(hh * DH, (hh + 1) * DH)
        in_maps.append(
            {
                "xb": np.ascontiguousarray(x[b]),
                "wq": np.ascontiguousarray(Wq[:, sl]),
                "wk": np.ascontiguousarray(Wk[:, sl]),
                "wv": np.ascontiguousarray(Wv[:, sl]),
                "wo": np.ascontiguousarray(Wo[sl, :]),
                "bqc": np.ascontiguousarray(bq[sl].reshape(4, P).T),
                "bkr": np.ascontiguousarray(np.broadcast_to(bk[sl], (P, DH))),
                "bvr": np.ascontiguousarray(np.broadcast_to(bv[sl], (P, DH))),
                "E": E,
                "F": F,
                "ident": np.eye(P, dtype=np.float32),
            }
        )
    return in_maps


def _ensure_profile_hook():
    """The container's `antenv` stub lacks `axon_hooks`; synthesize it so
    run_bass_kernel_spmd(trace=True) can reach the NTFF capture ABI in
    libaxon_pjrt.so (see trn_agent_boot.trn_boot)."""
    import types
    import antenv

    if hasattr(antenv, "axon_hooks"):
        return
    mod = types.ModuleType("antenv.axon_hooks")
    _state = {"hook": None}
    mod.set_axon_ntff_profile_hook = lambda h: _state.__setitem__("hook", h)
    mod.get_axon_ntff_profile_hook = lambda: _state["hook"]
    sys.modules["antenv.axon_hooks"] = mod
    antenv.axon_hooks = mod
    try:
        from trn_agent_boot.trn_boot import _ntff_profile_via_ctypes

        mod.set_axon_ntff_profile_hook(
            _ntff_profile_via_ctypes("/opt/axon/libaxon_pjrt.so")
        )
    except Exception as e:
        print(f"profile hook setup failed: {e}", file=sys.stderr)


def run(inputs, trace=False, **kw):
    if trace:
        _ensure_profile_hook()
    nc = _get_program()
    in_maps = _shard_inputs(inputs)
    res = bass_utils.run_bass_kernel_spmd(
        nc, in_maps, core_ids=list(range(8)), trace=trace, **kw
    )
    bo = np.asarray(inputs["bo"], np.float32)
    x = np.asarray(inputs["x"], np.float32)
    Bc = x.shape[0]
    y = np.empty((Bc, L, D), np.float32)
    for b in range(Bc):
        y[b] = res.results[2 * b]["y"] + res.results[2 * b + 1]["y"] + bo
    return y, res


def kernel(**inputs):
    n_heads = int(inputs.get("n_heads", H))
    assert n_heads == H, f"kernel hardcoded for {H} heads, got {n_heads}"
    y, _ = run(inputs, trace=False)
    return y


# revision 13
# speedup vs baseline: 2.0050x; 2.0050x over previous
"""Linformer attention TRN2 kernel (8 NeuronCores).

Sharding: core c handles batch b = c//2 and head-half hh = c%2
(8 of 16 heads = 512 of 1024 feature columns of Wq/Wk/Wv, and the
matching 512 rows of Wo). Each core computes a partial output
y_part = attn_out_half @ Wo[hh*512:(hh+1)*512, :]; the host sums the
two partials per batch and adds bo.

Per-core dataflow (L=4096, D=1024, 8 heads x hd=64, K=256):
  phase 1 (stream L in slices of 256):
    xT   = transpose(x_slice)                       (PE transpose)
    qT  += Wq_s^T @ xT       -> [512, L]  (SBUF-resident, bias folded)
    k,v  = xT^T @ Wk_s/Wv_s  -> [Lslice, 512] (transient, bias folded)
    kET += k_headpair^T @ E  -> [128(2 heads), 256] x4   (PSUM-resident)
    vF  += F_chunk^T @ v     -> [128(Kchunk), 512(8 heads)] x2 (PSUM)
  phase 2 (per L-tile of 512, per head):
    scores = qT_h^T @ kET_h            [128L, 256]   (PSUM)
    m = rowmax; s = exp(scale*scores - scale*m), Z = rowsum (ACT accum)
    s = exp(scale*scores - scale*m - ln Z)           (normalized)
    sT = PE-transpose(s)               [256, 512L]
    outT_h = vF_h^T @ sT               [64, 512L]
    y_tile = outT^T @ Wo_s             [128L, 1024] -> DRAM
"""

import sys

sys.path.insert(0, "/opt/trn_rl_repo")

import numpy as np

import concourse.bass as bass
import concourse.mybir as mybir
import concourse.tile as tile
from concourse import bacc
from concourse import bass_utils

B, L, D, H, HD, K = 4, 4096, 1024, 16, 64, 256
DH = 512                      # per-core feature slice (8 heads x 64)
NHL = 8                       # heads per core
SCALE = HD ** -0.5
P = 128
LS1 = 256                     # phase-1 L slice
NS1 = L // LS1                # 16 phase-1 iterations
LT2 = 512                     # phase-2 L tile
NT2 = L // LT2                # 8 phase-2 iterations
F32 = mybir.dt.float32
F32R = mybir.dt.float32r

USE_F32R = True               # fast fp32 matmul mode for GEMM operands
MMDT = F32R if USE_F32R else F32   # dtype of every matmul-feeding tensor

_CACHE = {}


def _r(ap):
    return ap


def _rt(ap):
    return ap


def build_program():
    nc = bacc.Bacc("TRN2", target_bir_lowering=False, debug=False)

    xb = nc.dram_tensor("xb", [L, D], MMDT, kind="ExternalInput").ap()
    wq = nc.dram_tensor("wq", [D, DH], MMDT, kind="ExternalInput").ap()
    wk = nc.dram_tensor("wk", [D, DH], MMDT, kind="ExternalInput").ap()
    wv = nc.dram_tensor("wv", [D, DH], MMDT, kind="ExternalInput").ap()
    wo = nc.dram_tensor("wo", [DH, D], MMDT, kind="ExternalInput").ap()
    bqc = nc.dram_tensor("bqc", [P, DH // P], F32, kind="ExternalInput").ap()
    bkr = nc.dram_tensor("bkr", [P, DH], F32, kind="ExternalInput").ap()
    bvr = nc.dram_tensor("bvr", [P, DH], F32, kind="ExternalInput").ap()
    Edr = nc.dram_tensor("E", [L, K], MMDT, kind="ExternalInput").ap()
    Fdr = nc.dram_tensor("F", [L, K], MMDT, kind="ExternalInput").ap()
    idr = nc.dram_tensor("ident", [P, P], MMDT, kind="ExternalInput").ap()
    ydr = nc.dram_tensor("y", [L, D], F32, kind="ExternalOutput").ap()

    with tile.TileContext(nc) as tc:
        with (
            tc.tile_pool(name="const", bufs=1) as constp,
            tc.tile_pool(name="persist", bufs=1) as persist,
        ):
            ident_t = constp.tile([P, P], MMDT, name="ident_t")
            nc.sync.dma_start(ident_t[:], idr)
            ident = ident_t[:]
            bqc_sb = constp.tile([P, DH // P], F32)
            nc.sync.dma_start(bqc_sb[:], bqc)
            bkr_sb = constp.tile([P, DH], F32)
            nc.sync.dma_start(bkr_sb[:], bkr)
            bvr_sb = constp.tile([P, DH], F32)
            nc.sync.dma_start(bvr_sb[:], bvr)

            # persistent SBUF tensors
            qT = persist.tile([P, 4, L], MMDT)          # qT[c*128+p, l]
            kET_sb = persist.tile([P, 4, K], MMDT)      # [2-head pair rows, pair, k]
            vF_sb = persist.tile([P, 2, DH], MMDT)      # [k within chunk, kchunk, h*64+d]

            # ---------------- phase 1 ----------------
            with (
                tc.tile_pool(name="w1", bufs=1) as w1,
                tc.tile_pool(name="xin", bufs=2) as xin,
                tc.tile_pool(name="xtp", bufs=2) as xtp,
                tc.tile_pool(name="kvp", bufs=2) as kvp,
                tc.tile_pool(name="efp", bufs=2) as efp,
                tc.tile_pool(name="ps_acc", bufs=1, space="PSUM") as ps_acc,
                tc.tile_pool(name="ps_tp1", bufs=2, space="PSUM") as ps_tp1,
                tc.tile_pool(name="ps_mm1", bufs=2, space="PSUM") as ps_mm1,
            ):
                wq_sb = w1.tile([P, D // P, DH], MMDT, tag="wq")
                nc.sync.dma_start(wq_sb[:], wq.rearrange("(c p) n -> p c n", p=P))
                wk_sb = w1.tile([P, D // P, DH], MMDT, tag="wk")
                nc.sync.dma_start(wk_sb[:], wk.rearrange("(c p) n -> p c n", p=P))
                wv_sb = w1.tile([P, D // P, DH], MMDT, tag="wv")
                nc.sync.dma_start(wv_sb[:], wv.rearrange("(c p) n -> p c n", p=P))

                # persistent PSUM accumulators
                kET_ps = [
                    ps_acc.tile([P, 2 * K], F32, tag=f"kET{i}", name=f"kET_ps{i}")
                    for i in range(2)
                ]
                vF_ps = [
                    ps_acc.tile([P, DH], F32, tag=f"vF{i}", name=f"vF_ps{i}")
                    for i in range(2)
                ]

                for ls in range(NS1):
                    l0 = ls * LS1
                    x_sl = xin.tile([P, LS1 // P, D], MMDT)
                    nc.sync.dma_start(
                        x_sl[:], xb[l0 : l0 + LS1, :].rearrange("(a p) d -> p a d", p=P)
                    )
                    # transpose x slice -> xT [d, l]  (tile [128, 8 dchunks, 256])
                    xT = xtp.tile([P, D // P, LS1], MMDT)
                    for a in range(LS1 // P):
                        for g in range(2):  # 2 groups of 4 dchunks
                            tp = ps_tp1.tile([P, 4 * P], MMDT, tag="tp")
                            for dd in range(4):
                                dc = g * 4 + dd
                                nc.tensor.transpose(
                                    tp[:, dd * P : (dd + 1) * P],
                                    _rt(x_sl[:, a, dc * P : (dc + 1) * P]),
                                    _rt(ident),
                                )
                            # copy the 4 transposed [128,128] blocks into xT
                            nc.scalar.copy(
                                xT[:, g * 4 : (g + 1) * 4, a * P : (a + 1) * P],
                                tp[:].rearrange("p (c n) -> p c n", n=P),
                            )
                    # qT chunks: lhsT = wq chunk, rhs = xT
                    for rc in range(4):
                        pq = ps_mm1.tile([P, LS1], F32, tag="mm1")
                        for dc in range(D // P):
                            nc.tensor.matmul(
                                pq[:],
                                _r(wq_sb[:, dc, rc * P : (rc + 1) * P]),
                                _r(xT[:, dc, :]),
                                start=(dc == 0),
                                stop=(dc == D // P - 1),
                            )
                        nc.vector.tensor_scalar_add(
                            qT[:, rc, l0 : l0 + LS1], pq[:], bqc_sb[:, rc : rc + 1]
                        )
                    # k, v natural layout slices
                    k_sl = kvp.tile([P, LS1 // P, DH], MMDT, tag="k")
                    v_sl = kvp.tile([P, LS1 // P, DH], MMDT, tag="v")
                    for a in range(LS1 // P):
                        pk = ps_mm1.tile([P, DH], F32, tag="mm1")
                        for dc in range(D // P):
                            nc.tensor.matmul(
                                pk[:],
                                _r(xT[:, dc, a * P : (a + 1) * P]),
                                _r(wk_sb[:, dc, :]),
                                start=(dc == 0),
                                stop=(dc == D // P - 1),
                            )
                        nc.vector.tensor_add(k_sl[:, a, :], pk[:], bkr_sb[:])
                        pv = ps_mm1.tile([P, DH], F32, tag="mm1")
                        for dc in range(D // P):
                            nc.tensor.matmul(
                                pv[:],
                                _r(xT[:, dc, a * P : (a + 1) * P]),
                                _r(wv_sb[:, dc, :]),
                                start=(dc == 0),
                                stop=(dc == D // P - 1),
                            )
                        nc.vector.tensor_add(v_sl[:, a, :], pv[:], bvr_sb[:])
                    # E / F slices
                    e_sl = efp.tile([P, LS1 // P, K], MMDT, tag="E")
                    nc.sync.dma_start(
                        e_sl[:], Edr[l0 : l0 + LS1, :].rearrange("(a p) k -> p a k", p=P)
                    )
                    f_sl = efp.tile([P, LS1 // P, K], MMDT, tag="F")
                    nc.sync.dma_start(
                        f_sl[:], Fdr[l0 : l0 + LS1, :].rearrange("(a p) k -> p a k", p=P)
                    )
                    first = ls == 0
                    last = ls == NS1 - 1
                    for a in range(LS1 // P):
                        # kET accumulation: 4 head-pairs, 2 pairs share a
                        # PSUM bank -> only one start/stop per bank (zero
                        # region); first write per element overwrites.
                        for pr in range(4):
                            nc.tensor.matmul(
                                kET_ps[pr // 2][:, (pr % 2) * K : (pr % 2 + 1) * K],
                                _r(k_sl[:, a, pr * P : (pr + 1) * P]),
                                _r(e_sl[:, a, :]),
                                start=(first and a == 0 and pr % 2 == 0),
                                stop=(last and a == LS1 // P - 1 and pr % 2 == 1),
                            )
                        # vF accumulation: 2 K-chunks
                        for kc in range(2):
                            nc.tensor.matmul(
                                vF_ps[kc][:],
                                _r(f_sl[:, a, kc * P : (kc + 1) * P]),
                                _r(v_sl[:, a, :]),
                                start=(first and a == 0),
                                stop=(last and a == LS1 // P - 1),
                            )
                # copy accumulators to SBUF
                for i in range(2):
                    nc.scalar.copy(
                        kET_sb[:, 2 * i : 2 * i + 2, :],
                        kET_ps[i][:].rearrange("p (c n) -> p c n", n=K),
                    )
                    nc.scalar.copy(vF_sb[:, i, :], vF_ps[i][:])

            # ---------------- phase 2 ----------------
            with (
                tc.tile_pool(name="w2", bufs=1) as w2,
                tc.tile_pool(name="sp", bufs=3) as sp,
                tc.tile_pool(name="stp", bufs=2) as stp,
                tc.tile_pool(name="otp", bufs=2) as otp,
                tc.tile_pool(name="obp", bufs=2) as obp,
                tc.tile_pool(name="yp", bufs=2) as yp,
                tc.tile_pool(name="stat", bufs=8) as stat,
                tc.tile_pool(name="ps_s", bufs=2, space="PSUM") as ps_s,
                tc.tile_pool(name="ps_st", bufs=2, space="PSUM") as ps_st,
                tc.tile_pool(name="ps_pv", bufs=2, space="PSUM") as ps_pv,
                tc.tile_pool(name="ps_y", bufs=2, space="PSUM") as ps_y,
            ):
                wo_sb = w2.tile([P, DH // P, D], MMDT)
                nc.sync.dma_start(wo_sb[:], wo.rearrange("(c p) n -> p c n", p=P))

                for lt in range(NT2):
                    l0 = lt * LT2
                    outT = otp.tile([P, 4, LT2], MMDT)
                    for h in range(NHL):
                        po = (h % 2) * HD          # partition offset within pair
                        pair = h // 2
                        sT = stp.tile([P, 2, LT2], MMDT)
                        for j in range(LT2 // P):
                            ps = ps_s.tile([P, K], F32)
                            nc.tensor.matmul(
                                ps[:],
                                _r(qT[po : po + HD, pair, l0 + j * P : l0 + (j + 1) * P]),
                                _r(kET_sb[po : po + HD, pair, :]),
                                start=True,
                                stop=True,
                            )
                            m = stat.tile([P, 1], F32, tag="m")
                            nc.vector.reduce_max(m[:], ps[:], axis=mybir.AxisListType.X)
                            negm = stat.tile([P, 1], F32, tag="negm")
                            nc.vector.tensor_scalar_mul(negm[:], m[:], -SCALE)
                            z = stat.tile([P, 1], F32, tag="z")
                            s = sp.tile([P, K], MMDT)
                            nc.scalar.activation(
                                s[:],
                                ps[:],
                                mybir.ActivationFunctionType.Exp,
                                bias=negm[:],
                                scale=SCALE,
                                accum_out=z[:],
                            )
                            rz = stat.tile([P, 1], F32, tag="rz")
                            nc.vector.reciprocal(rz[:], z[:])
                            nc.vector.tensor_scalar_mul(s[:], s[:], rz[:])
                            # transpose s [128, 256] -> two [128, 128] blocks
                            pt = ps_st.tile([P, 2 * P], MMDT, tag="st")
                            for kc in range(2):
                                nc.tensor.transpose(
                                    pt[:, kc * P : (kc + 1) * P],
                                    _rt(s[:, kc * P : (kc + 1) * P]),
                                    _rt(ident),
                                )
                            eng = nc.vector if j % 2 == 0 else nc.scalar
                            if j % 2 == 0:
                                nc.vector.tensor_copy(
                                    sT[:, :, j * P : (j + 1) * P],
                                    pt[:].rearrange("p (c n) -> p c n", n=P),
                                )
                            else:
                                nc.scalar.copy(
                                    sT[:, :, j * P : (j + 1) * P],
                                    pt[:].rearrange("p (c n) -> p c n", n=P),
                                )
                        # PV: outT_h [64, LT2] (PSUM dst must start at partition 0)
                        pp = ps_pv.tile([HD, LT2], F32)
                        for kc in range(2):
                            nc.tensor.matmul(
                                pp[:],
                                _r(vF_sb[:, kc, h * HD : (h + 1) * HD]),
                                _r(sT[:, kc, :]),
                                start=(kc == 0),
                                stop=(kc == 1),
                            )
                        if po == 0:
                            nc.scalar.copy(outT[0:HD, pair, :], pp[:])
                        else:
                            ob = obp.tile([HD, LT2], MMDT, name="obounce")
                            nc.scalar.copy(ob[:], pp[:])
                            nc.sync.dma_start(outT[po : po + HD, pair, :], ob[:])
                    # y = outT^T @ Wo_s
                    for j in range(LT2 // P):
                        y_sb = yp.tile([P, D], F32)
                        for nh in range(2):
                            py = ps_y.tile([P, DH], F32)
                            for c in range(4):
                                nc.tensor.matmul(
                                    py[:],
                                    _r(outT[:, c, j * P : (j + 1) * P]),
                                    _r(wo_sb[:, c, nh * DH : (nh + 1) * DH]),
                                    start=(c == 0),
                                    stop=(c == 3),
                                )
                            nc.scalar.copy(y_sb[:, nh * DH : (nh + 1) * DH], py[:])
                        nc.sync.dma_start(ydr[l0 + j * P : l0 + (j + 1) * P, :], y_sb[:])
    nc.compile()
    return nc


def _get_program():
    if "nc" not in _CACHE:
        _CACHE["nc"] = build_program()
    return _CACHE["nc"]


def _shard_inputs(inputs):
    x = np.asarray(inputs["x"], np.float32)
    Wq = np.asarray(inputs["Wq"], np.float32)
    bq = np.asarray(inputs["bq"], np.float32)
    Wk = np.asarray(inputs["Wk"], np.float32)
    bk = np.asarray(inputs["bk"], np.float32)
    Wv = np.asarray(inputs["Wv"], np.float32)
    bv = np.asarray(inputs["bv"], np.float32)
    E = np.ascontiguousarray(np.asarray(inputs["E"], np.float32))
    F = np.ascontiguousarray(np.asarray(inputs["F"], np.float32))
    Wo = np.asarray(inputs["Wo"], np.float32)
    in_maps = []
    for c in range(8):
        b, hh = c // 2, c % 2
        sl = slice(hh * DH, (hh + 1) * DH)
        in_maps.append(
            {
                "xb": np.ascontiguousarray(x[b]),
                "wq": np.ascontiguousarray(Wq[:, sl]),
                "wk": np.ascontiguousarray(Wk[:, sl]),
                "wv": np.ascontiguousarray(Wv[:, sl]),
                "wo": np.ascontiguousarray(Wo[sl, :]),
                "bqc": np.ascontiguousarray(bq[sl].reshape(4, P).T),
                "bkr": np.ascontiguousarray(np.broadcast_to(bk[sl], (P, DH))),
                "bvr": np.ascontiguousarray(np.broadcast_to(bv[sl], (P, DH))),
                "E": E,
                "F": F,
                "ident": np.eye(P, dtype=np.float32),
            }
        )
    return in_maps


def _ensure_profile_hook():
    """The container's `antenv` stub lacks `axon_hooks`; synthesize it so
    run_bass_kernel_spmd(trace=True) can reach the NTFF capture ABI in
    libaxon_pjrt.so (see trn_agent_boot.trn_boot)."""
    import types
    import antenv

    if hasattr(antenv, "axon_hooks"):
        return
    mod = types.ModuleType("antenv.axon_hooks")
    _state = {"hook": None}
    mod.set_axon_ntff_profile_hook = lambda h: _state.__setitem__("hook", h)
    mod.get_axon_ntff_profile_hook = lambda: _state["hook"]
    sys.modules["antenv.axon_hooks"] = mod
    antenv.axon_hooks = mod
    try:
        from trn_agent_boot.trn_boot import _ntff_profile_via_ctypes

        mod.set_axon_ntff_profile_hook(
            _ntff_profile_via_ctypes("/opt/axon/libaxon_pjrt.so")
        )
    except Exception as e:
        print(f"profile hook setup failed: {e}", file=sys.stderr)


def run(inputs, trace=False, **kw):
    if trace:
        _ensure_profile_hook()
    nc = _get_program()
    in_maps = _shard_inputs(inputs)
    res = bass_utils.run_bass_kernel_spmd(
        nc, in_maps, core_ids=list(range(8)), trace=trace, **kw
    )
    bo = np.asarray(inputs["bo"], np.float32)
    x = np.asarray(inputs["x"], np.float32)
    Bc = x.shape[0]
    y = np.empty((Bc, L, D), np.float32)
    for b in range(Bc):
        y[b] = res.results[2 * b]["y"] + res.results[2 * b + 1]["y"] + bo
    return y, res


def kernel(**inputs):
    n_heads = int(inputs.get("n_heads", H))
    assert n_heads == H, f"kernel hardcoded for {H} heads, got {n_heads}"
    y, _ = run(inputs, trace=False)
    return y


# revision 16
# speedup vs baseline: 2.3112x; 1.1527x over previous
"""Linformer attention TRN2 kernel (8 NeuronCores).

Sharding: core c handles batch b = c//2 and head-half hh = c%2
(8 of 16 heads = 512 of 1024 feature columns of Wq/Wk/Wv, and the
matching 512 rows of Wo). Each core computes a partial output
y_part = attn_out_half @ Wo[hh*512:(hh+1)*512, :]; the host sums the
two partials per batch and adds bo.

Per-core dataflow (L=4096, D=1024, 8 heads x hd=64, K=256):
  phase 1 (stream L in slices of 256):
    xT   = transpose(x_slice)                       (PE transpose)
    qT  += Wq_s^T @ xT       -> [512, L]  (SBUF-resident, bias folded)
    k,v  = xT^T @ Wk_s/Wv_s  -> [Lslice, 512] (transient, bias folded)
    kET += k_headpair^T @ E  -> [128(2 heads), 256] x4   (PSUM-resident)
    vF  += F_chunk^T @ v     -> [128(Kchunk), 512(8 heads)] x2 (PSUM)
  phase 2 (per L-tile of 512, per head):
    scores = qT_h^T @ kET_h            [128L, 256]   (PSUM)
    m = rowmax; s = exp(scale*scores - scale*m), Z = rowsum (ACT accum)
    s = exp(scale*scores - scale*m - ln Z)           (normalized)
    sT = PE-transpose(s)               [256, 512L]
    outT_h = vF_h^T @ sT               [64, 512L]
    y_tile = outT^T @ Wo_s             [128L, 1024] -> DRAM
"""

import sys

sys.path.insert(0, "/opt/trn_rl_repo")

import numpy as np
import ml_dtypes

import concourse.bass as bass
import concourse.mybir as mybir
import concourse.tile as tile
from concourse import bacc
from concourse import bass_utils

B, L, D, H, HD, K = 4, 4096, 1024, 16, 64, 256
DH = 512                      # per-core feature slice (8 heads x 64)
NHL = 8                       # heads per core
SCALE = HD ** -0.5
P = 128
LS1 = 512                     # phase-1 L slice
NS1 = L // LS1                # 16 phase-1 iterations
LT2 = 512                     # phase-2 L tile
NT2 = L // LT2                # 8 phase-2 iterations
F32 = mybir.dt.float32
F32R = mybir.dt.float32r
BF16 = mybir.dt.bfloat16

USE_F32R = True               # fast fp32 matmul mode for GEMM operands
MMDT = F32R if USE_F32R else F32   # dtype of every matmul-feeding tensor

_CACHE = {}


def _r(ap):
    return ap


def _rt(ap):
    return ap


def build_program():
    nc = bacc.Bacc("TRN2", target_bir_lowering=False, debug=False)

    xt = nc.dram_tensor("xt", [D, L], MMDT, kind="ExternalInput").ap()
    wq = nc.dram_tensor("wq", [D, DH], MMDT, kind="ExternalInput").ap()
    wk = nc.dram_tensor("wk", [D, DH], MMDT, kind="ExternalInput").ap()
    wv = nc.dram_tensor("wv", [D, DH], MMDT, kind="ExternalInput").ap()
    wo = nc.dram_tensor("wo", [DH, D], MMDT, kind="ExternalInput").ap()
    bqc = nc.dram_tensor("bqc", [P, DH // P], F32, kind="ExternalInput").ap()
    bkr = nc.dram_tensor("bkr", [P, DH], F32, kind="ExternalInput").ap()
    bvr = nc.dram_tensor("bvr", [P, DH], F32, kind="ExternalInput").ap()
    Edr = nc.dram_tensor("E", [L, K], MMDT, kind="ExternalInput").ap()
    Fdr = nc.dram_tensor("F", [L, K], MMDT, kind="ExternalInput").ap()
    idr = nc.dram_tensor("ident", [P, P], MMDT, kind="ExternalInput").ap()
    idbr = nc.dram_tensor("identb", [P, P], BF16, kind="ExternalInput").ap()
    ydr = nc.dram_tensor("y", [L, D], F32, kind="ExternalOutput").ap()

    with tile.TileContext(nc) as tc:
        with (
            tc.tile_pool(name="const", bufs=1) as constp,
            tc.tile_pool(name="persist", bufs=1) as persist,
        ):
            ident_t = constp.tile([P, P], MMDT, name="ident_t")
            nc.sync.dma_start(ident_t[:], idr)
            ident = ident_t[:]
            identb_t = constp.tile([P, P], BF16, name="identb_t")
            nc.sync.dma_start(identb_t[:], idbr)
            identb = identb_t[:]
            bqc_sb = constp.tile([P, DH // P], F32)
            nc.sync.dma_start(bqc_sb[:], bqc)
            bkr_sb = constp.tile([P, DH], F32)
            nc.sync.dma_start(bkr_sb[:], bkr)
            bvr_sb = constp.tile([P, DH], F32)
            nc.sync.dma_start(bvr_sb[:], bvr)

            # persistent SBUF tensors
            qT = persist.tile([P, 4, L], MMDT)          # qT[c*128+p, l]
            kET_sb = persist.tile([P, 4, K], MMDT)      # [2-head pair rows, pair, k]
            vF_sb = persist.tile([P, 2, DH], BF16)      # [k within chunk, kchunk, h*64+d]

            # ---------------- phase 1 ----------------
            with (
                tc.tile_pool(name="w1", bufs=1) as w1,
                tc.tile_pool(name="xtp", bufs=2) as xtp,
                tc.tile_pool(name="kvp", bufs=1) as kvp,
                tc.tile_pool(name="efp", bufs=2) as efp,
                tc.tile_pool(name="ps_acc", bufs=1, space="PSUM") as ps_acc,
                tc.tile_pool(name="ps_mm1", bufs=4, space="PSUM") as ps_mm1,
            ):
                wq_sb = w1.tile([P, D // P, DH], MMDT, tag="wq")
                nc.sync.dma_start(wq_sb[:], wq.rearrange("(c p) n -> p c n", p=P))
                wk_sb = w1.tile([P, D // P, DH], MMDT, tag="wk")
                nc.sync.dma_start(wk_sb[:], wk.rearrange("(c p) n -> p c n", p=P))
                wv_sb = w1.tile([P, D // P, DH], MMDT, tag="wv")
                nc.sync.dma_start(wv_sb[:], wv.rearrange("(c p) n -> p c n", p=P))

                # persistent PSUM accumulators
                kET_ps = [
                    ps_acc.tile([P, 2 * K], F32, tag=f"kET{i}", name=f"kET_ps{i}")
                    for i in range(2)
                ]
                vF_ps = [
                    ps_acc.tile([P, DH], F32, tag=f"vF{i}", name=f"vF_ps{i}")
                    for i in range(2)
                ]

                for ls in range(NS1):
                    l0 = ls * LS1
                    # x^T slice [d, l] comes pre-transposed from the host
                    xT = xtp.tile([P, D // P, LS1], MMDT)
                    nc.sync.dma_start(
                        xT[:], xt[:, l0 : l0 + LS1].rearrange("(c p) l -> p c l", p=P)
                    )
                    # qT chunks: lhsT = wq chunk, rhs = xT
                    for rc in range(4):
                        pq = ps_mm1.tile([P, LS1], F32, tag="mm1")
                        for dc in range(D // P):
                            nc.tensor.matmul(
                                pq[:],
                                _r(wq_sb[:, dc, rc * P : (rc + 1) * P]),
                                _r(xT[:, dc, :]),
                                start=(dc == 0),
                                stop=(dc == D // P - 1),
                            )
                        nc.vector.tensor_scalar(
                            qT[:, rc, l0 : l0 + LS1],
                            pq[:],
                            bqc_sb[:, rc : rc + 1],
                            SCALE,
                            op0=mybir.AluOpType.add,
                            op1=mybir.AluOpType.mult,
                        )
                    # k, v natural layout slices
                    k_sl = kvp.tile([P, LS1 // P, DH], MMDT, tag="k")
                    v_sl = kvp.tile([P, LS1 // P, DH], MMDT, tag="v")
                    for a in range(LS1 // P):
                        pk = ps_mm1.tile([P, DH], F32, tag="mm1")
                        for dc in range(D // P):
                            nc.tensor.matmul(
                                pk[:],
                                _r(xT[:, dc, a * P : (a + 1) * P]),
                                _r(wk_sb[:, dc, :]),
                                start=(dc == 0),
                                stop=(dc == D // P - 1),
                            )
                        nc.vector.tensor_add(k_sl[:, a, :], pk[:], bkr_sb[:])
                        pv = ps_mm1.tile([P, DH], F32, tag="mm1")
                        for dc in range(D // P):
                            nc.tensor.matmul(
                                pv[:],
                                _r(xT[:, dc, a * P : (a + 1) * P]),
                                _r(wv_sb[:, dc, :]),
                                start=(dc == 0),
                                stop=(dc == D // P - 1),
                            )
                        nc.vector.tensor_add(v_sl[:, a, :], pv[:], bvr_sb[:])
                    # E / F slices
                    e_sl = efp.tile([P, LS1 // P, K], MMDT, tag="E")
                    nc.sync.dma_start(
                        e_sl[:], Edr[l0 : l0 + LS1, :].rearrange("(a p) k -> p a k", p=P)
                    )
                    f_sl = efp.tile([P, LS1 // P, K], MMDT, tag="F")
                    nc.sync.dma_start(
                        f_sl[:], Fdr[l0 : l0 + LS1, :].rearrange("(a p) k -> p a k", p=P)
                    )
                    first = ls == 0
                    last = ls == NS1 - 1
                    for a in range(LS1 // P):
                        # kET accumulation: 4 head-pairs, 2 pairs share a
                        # PSUM bank -> only one start/stop per bank (zero
                        # region); first write per element overwrites.
                        for pr in range(4):
                            nc.tensor.matmul(
                                kET_ps[pr // 2][:, (pr % 2) * K : (pr % 2 + 1) * K],
                                _r(k_sl[:, a, pr * P : (pr + 1) * P]),
                                _r(e_sl[:, a, :]),
                                start=(first and a == 0 and pr % 2 == 0),
                                stop=(last and a == LS1 // P - 1 and pr % 2 == 1),
                            )
                        # vF accumulation: 2 K-chunks
                        for kc in range(2):
                            nc.tensor.matmul(
                                vF_ps[kc][:],
                                _r(f_sl[:, a, kc * P : (kc + 1) * P]),
                                _r(v_sl[:, a, :]),
                                start=(first and a == 0),
                                stop=(last and a == LS1 // P - 1),
                            )
                # copy accumulators to SBUF
                for i in range(2):
                    nc.scalar.copy(
                        kET_sb[:, 2 * i : 2 * i + 2, :],
                        kET_ps[i][:].rearrange("p (c n) -> p c n", n=K),
                    )
                    nc.scalar.copy(vF_sb[:, i, :], vF_ps[i][:])

            # ---------------- phase 2 ----------------
            with (
                tc.tile_pool(name="w2", bufs=1) as w2,
                tc.tile_pool(name="sp", bufs=10) as sp,
                tc.tile_pool(name="stp", bufs=3) as stp,
                tc.tile_pool(name="otp", bufs=2) as otp,
                tc.tile_pool(name="obp", bufs=4) as obp,
                tc.tile_pool(name="yp", bufs=2) as yp,
                tc.tile_pool(name="stat", bufs=24) as stat,
                tc.tile_pool(name="ps_s", bufs=3, space="PSUM") as ps_s,
                tc.tile_pool(name="ps_st", bufs=2, space="PSUM") as ps_st,
                tc.tile_pool(name="ps_out", bufs=3, space="PSUM") as ps_out,
            ):
                wo_sb = w2.tile([P, DH // P, D], MMDT)
                nc.sync.dma_start(wo_sb[:], wo.rearrange("(c p) n -> p c n", p=P))

                for lt in range(NT2):
                    l0 = lt * LT2
                    outT = otp.tile([P, 4, LT2], MMDT)
                    for h in range(NHL):
                        po = (h % 2) * HD          # partition offset within pair
                        pair = h // 2
                        sT = stp.tile([P, 2, LT2], BF16)
                        for j in range(LT2 // P):
                            ps = ps_s.tile([P, K], F32)
                            nc.tensor.matmul(
                                ps[:],
                                _r(qT[po : po + HD, pair, l0 + j * P : l0 + (j + 1) * P]),
                                _r(kET_sb[po : po + HD, pair, :]),
                                start=True,
                                stop=True,
                            )
                            negm = stat.tile([P, 1], F32, tag="negm")
                            nc.vector.reduce_max(
                                negm[:], ps[:], axis=mybir.AxisListType.X, negate=True
                            )
                            z = stat.tile([P, 1], F32, tag="z")
                            s = sp.tile([P, K], BF16)
                            nc.scalar.activation(
                                s[:],
                                ps[:],
                                mybir.ActivationFunctionType.Exp,
                                bias=negm[:],
                                scale=1.0,
                                accum_out=z[:],
                            )
                            rz = stat.tile([P, 1], F32, tag="rz")
                            nc.vector.reciprocal(rz[:], z[:])
                            nc.vector.tensor_scalar_mul(s[:], s[:], rz[:])
                            # transpose s [128, 256] -> two [128, 128] blocks
                            pt = ps_st.tile([P, 2 * P], BF16, tag="st")
                            for kc in range(2):
                                nc.tensor.transpose(
                                    pt[:, kc * P : (kc + 1) * P],
                                    _rt(s[:, kc * P : (kc + 1) * P]),
                                    _rt(identb),
                                )
                            eng = nc.vector if j % 2 == 0 else nc.scalar
                            if j % 2 == 0:
                                nc.vector.tensor_copy(
                                    sT[:, :, j * P : (j + 1) * P],
                                    pt[:].rearrange("p (c n) -> p c n", n=P),
                                )
                            else:
                                nc.scalar.copy(
                                    sT[:, :, j * P : (j + 1) * P],
                                    pt[:].rearrange("p (c n) -> p c n", n=P),
                                )
                        # PV: outT_h [64, LT2] (PSUM dst must start at partition 0)
                        pp = ps_out.tile([HD, LT2], F32, tag="pvy", name="pp")
                        for kc in range(2):
                            nc.tensor.matmul(
                                pp[:],
                                _r(vF_sb[:, kc, h * HD : (h + 1) * HD]),
                                _r(sT[:, kc, :]),
                                start=(kc == 0),
                                stop=(kc == 1),
                            )
                        if po == 0:
                            nc.scalar.copy(outT[0:HD, pair, :], pp[:])
                        else:
                            ob = obp.tile([HD, LT2], MMDT, name="obounce")
                            nc.scalar.copy(ob[:], pp[:])
                            nc.sync.dma_start(outT[po : po + HD, pair, :], ob[:])
                    # y = outT^T @ Wo_s
                    for j in range(LT2 // P):
                        y_sb = yp.tile([P, D], F32)
                        for nh in range(2):
                            py = ps_out.tile([P, DH], F32, tag="pvy", name="py")
                            for c in range(4):
                                nc.tensor.matmul(
                                    py[:],
                                    _r(outT[:, c, j * P : (j + 1) * P]),
                                    _r(wo_sb[:, c, nh * DH : (nh + 1) * DH]),
                                    start=(c == 0),
                                    stop=(c == 3),
                                )
                            nc.scalar.copy(y_sb[:, nh * DH : (nh + 1) * DH], py[:])
                        nc.sync.dma_start(ydr[l0 + j * P : l0 + (j + 1) * P, :], y_sb[:])
    nc.compile()
    return nc


def _get_program():
    if "nc" not in _CACHE:
        _CACHE["nc"] = build_program()
    return _CACHE["nc"]


def _shard_inputs(inputs):
    x = np.asarray(inputs["x"], np.float32)
    Wq = np.asarray(inputs["Wq"], np.float32)
    bq = np.asarray(inputs["bq"], np.float32)
    Wk = np.asarray(inputs["Wk"], np.float32)
    bk = np.asarray(inputs["bk"], np.float32)
    Wv = np.asarray(inputs["Wv"], np.float32)
    bv = np.asarray(inputs["bv"], np.float32)
    E = np.ascontiguousarray(np.asarray(inputs["E"], np.float32))
    F = np.ascontiguousarray(np.asarray(inputs["F"], np.float32))
    Wo = np.asarray(inputs["Wo"], np.float32)
    in_maps = []
    for c in range(8):
        b, hh = c // 2, c % 2
        sl = slice(hh * DH, (hh + 1) * DH)
        in_maps.append(
            {
                "xt": np.ascontiguousarray(x[b].T),
                "wq": np.ascontiguousarray(Wq[:, sl]),
                "wk": np.ascontiguousarray(Wk[:, sl]),
                "wv": np.ascontiguousarray(Wv[:, sl]),
                "wo": np.ascontiguousarray(Wo[sl, :]),
                "bqc": np.ascontiguousarray(bq[sl].reshape(4, P).T),
                "bkr": np.ascontiguousarray(np.broadcast_to(bk[sl], (P, DH))),
                "bvr": np.ascontiguousarray(np.broadcast_to(bv[sl], (P, DH))),
                "E": E,
                "F": F,
                "ident": np.eye(P, dtype=np.float32),
                "identb": np.eye(P, dtype=ml_dtypes.bfloat16),
            }
        )
    return in_maps


def _ensure_profile_hook():
    """The container's `antenv` stub lacks `axon_hooks`; synthesize it so
    run_bass_kernel_spmd(trace=True) can reach the NTFF capture ABI in
    libaxon_pjrt.so (see trn_agent_boot.trn_boot)."""
    import types
    import antenv

    if hasattr(antenv, "axon_hooks"):
        return
    mod = types.ModuleType("antenv.axon_hooks")
    _state = {"hook": None}
    mod.set_axon_ntff_profile_hook = lambda h: _state.__setitem__("hook", h)
    mod.get_axon_ntff_profile_hook = lambda: _state["hook"]
    sys.modules["antenv.axon_hooks"] = mod
    antenv.axon_hooks = mod
    try:
        from trn_agent_boot.trn_boot import _ntff_profile_via_ctypes

        mod.set_axon_ntff_profile_hook(
            _ntff_profile_via_ctypes("/opt/axon/libaxon_pjrt.so")
        )
    except Exception as e:
        print(f"profile hook setup failed: {e}", file=sys.stderr)


def run(inputs, trace=False, **kw):
    if trace:
        _ensure_profile_hook()
    nc = _get_program()
    in_maps = _shard_inputs(inputs)
    res = bass_utils.run_bass_kernel_spmd(
        nc, in_maps, core_ids=list(range(8)), trace=trace, **kw
    )
    bo = np.asarray(inputs["bo"], np.float32)
    x = np.asarray(inputs["x"], np.float32)
    Bc = x.shape[0]
    y = np.empty((Bc, L, D), np.float32)
    for b in range(Bc):
        y[b] = res.results[2 * b]["y"] + res.results[2 * b + 1]["y"] + bo
    return y, res


def kernel(**inputs):
    n_heads = int(inputs.get("n_heads", H))
    assert n_heads == H, f"kernel hardcoded for {H} heads, got {n_heads}"
    y, _ = run(inputs, trace=False)
    return y
